# revision 39
# baseline (speedup 1.0000x reference)
"""ConvergedInhibition TRN2 kernel.

The reference computes, per pixel (n,h,w), an FFT deconvolution along the
channel axis: y = ifft(fft(x)/fft(k)).real. Since k is fixed, this is a
circular convolution with g = ifft(1/fft(k)): y[i] = sum_j g[(i-j) mod C] x[j]
— a dense CxC circulant matmul applied to every pixel. Viewing activations[n]
as a [C, H*W] matrix A_n, the problem is out_n = G @ A_n: a [512,512] x
[512,3136] matmul per image, data-parallel over 32 images across 8 cores.

Implementation choices (measured on HW):
- The deconv kernel g is concentrated in a ~224-wide circular window.
  Rotating output rows by S=288 (z[r] = y[(r+S) mod C]) aligns the support
  so only 2 of 4 K-chunks of the contraction carry mass (each output row
  keeps a 256-wide sliding window of g; truncation costs ~2e-3 rel).
  The rotation is undone by a host-side gather.
- fp8 (e3m4) I/O: |x| < 6 << 15.5 = e3m4 max, 4 mantissa bits -> ~1.34e-2
  rms rounding per side (measured end-to-end rel err 1.907e-2, HW matches
  the numpy simulation exactly). Per-zc output dtype stays configurable.
  Weights are fp16 (PE upcasts operands to FP22, mixed dtypes allowed).
- Only the 8 needed [128,128] weight blocks ship (256 KB, one DMA on the
  otherwise-idle scalar ring).
- Each dma_start occupies its HWDGE ring ~630ns regardless of size, so
  DMAs are as large as possible: the host pre-arranges activations
  partition-major/cb-major so each (img, cb) load is one fully contiguous
  400KB 2D transfer; stores are half-image-width. Loads + most stores ride
  the sync ring; the last image's zc3 stores drain on the scalar ring.
- Startup: the first (img0, cb0) load is prefetched from the semaphore-
  clears block in three pieces (jc{0,1} | jc2 | jc3) and that block's tile
  order is zc [1,2,3,0], so real matmuls start on the first piece while
  the rest is in flight. Junk warmup matmuls bridge from the clears block
  to first data so the PE's HAM clock gate (1.2GHz when cold, ~3.4us of
  activity to lift) is fully open before real work; without this, any
  >2.5us PE idle gap re-arms the throttle mid-kernel.
- PSUM->out casts are split between DVE (zc 0,1) and ACT (zc 2,3), each
  with its own completion semaphore (PSUM bank = seq position % 8; the
  bank-reuse wait looks up the engine of the tile 8 positions back), so
  cast rate keeps up with the PE.
- Measured engine budget per core: PE window 42.6us (256 x 166ns, zero
  stalls), DVE 64 casts x 558ns, ACT 64 x 587ns, ~13.1MB HBM traffic.
"""

import numpy as np
import ml_dtypes

import concourse.bass as bass  # noqa: F401  (registers bass types)
import concourse.mybir as mybir
from concourse import bacc
from concourse.bass_utils import run_bass_kernel_spmd

N_CORES = 8
N, C, H, W = 32, 512, 56, 56
HW = H * W                      # 3136
IMGS = N // N_CORES             # 4 images per core
P = 128                         # partitions
NCHUNK = C // P                 # 4
PT = 392                        # pixel tile (free dim), 3136 = 8*392
NPT = HW // PT                  # 8
CB = 784                        # column block, 3136 = 4*784
NCB = HW // CB                  # 4
ROT = 288                       # output-row rotation aligning g's support
KEPT_D = (0, 1)                 # kept (zc - jc) mod 4 chunk distances
NKEPT = len(KEPT_D)

F8 = mybir.dt.float8e3
F16 = mybir.dt.float16
NP_OF = {F8: ml_dtypes.float8_e3m4, F16: np.float16}
A_DT = F8                       # activation (input) dtype
OUT_DT = [F8, F8, F8, F8]       # output dtype per output chunk zc
W_DT = F16                      # gt weights
CAST_ENG = ["v", "v", "a", "a"]  # cast engine per zc (v=DVE, a=ACT)

P2 = NPT // NCB                       # p-tiles per column block (2)
TILES_PER_CB = NCHUNK * P2            # 8 psum tiles per (img, cb)
TILES_PER_IMG = NCB * TILES_PER_CB    # 32
NTILES = IMGS * TILES_PER_IMG         # 128


# Tile execution sequence: position in TILE_SEQ is the tile id (PSUM bank =
# pos % 8, s_mm counts positions). (img0, cb0) runs zc order [1,2,3,0] so
# compute can start on the first half-load (zc1 reads only jc{0,1}; zc2/zc3
# only need the second half; zc0 wraps to jc3).
def _zc_order(img, cb):
    # (0,0): follow the three-piece first load (zc1 needs only jc{0,1}...).
    # Elsewhere: alternate the cast engines (v,v,a,a,v,v,a,a) so both
    # engines' cast chains pipeline with the matmul stream instead of one
    # engine's four casts bunching up after the last matmul of the block.
    return (1, 2, 3, 0) if (img, cb) == (0, 0) else (0, 2, 1, 3)


TILE_SEQ = []                      # (img, cb, zc, p2) in execution order
for _img in range(IMGS):
    for _cb in range(NCB):
        for _zc in _zc_order(_img, _cb):
            for _p2 in range(P2):
                TILE_SEQ.append((_img, _cb, _zc, _p2))
POS = {k: q for q, k in enumerate(TILE_SEQ)}

# cnt_eng[e][q]: number of tiles at positions <= q cast by engine e
_cnt = {"v": [0] * NTILES, "a": [0] * NTILES}
_c = {"v": 0, "a": 0}
for _q, (_i, _b, _zc, _p) in enumerate(TILE_SEQ):
    _c[CAST_ENG[_zc]] += 1
    _cnt["v"][_q] = _c["v"]
    _cnt["a"][_q] = _c["a"]

_CACHE = {}


def _build_nc():
    nc = bacc.Bacc("TRN2", target_bir_lowering=False, debug=False,
                   num_devices=N_CORES)
    # host pre-arranges activations partition-major and cb-major
    # ([img, p, cb, jc, m] flattened) so every (img, cb) load is a single
    # 2D DMA with 3136B-contiguous runs on both the HBM and SBUF side
    act = nc.dram_tensor("act", [IMGS, P, NCHUNK * HW], A_DT,
                         kind="ExternalInput")
    gtp = nc.dram_tensor("gtp", [P, NCHUNK * NKEPT * P], W_DT,
                         kind="ExternalInput")
    out = [nc.dram_tensor(f"out{zc}", [IMGS, P, HW], OUT_DT[zc],
                          kind="ExternalOutput") for zc in range(NCHUNK)]

    CBW = NCHUNK * CB  # 3136 cols per (cb) block in the cb-major layout

    from contextlib import ExitStack
    with ExitStack() as ctx:
        a_sb = [ctx.enter_context(
            nc.sbuf_tensor(f"a_sb{h}", [P, NCHUNK * HW], A_DT)).ap()
            for h in range(2)]
        gt_sb = ctx.enter_context(
            nc.sbuf_tensor("gt_sb", [P, NCHUNK * NKEPT * P], W_DT)).ap()
        o_sb = [[ctx.enter_context(
            nc.sbuf_tensor(f"o_sb{i}_{z}", [P, HW], OUT_DT[z])).ap()
            for z in range(NCHUNK)] for i in range(IMGS)]
        psum = [ctx.enter_context(
            nc.psum_tensor(f"ps{i}", [P, 512], mybir.dt.float32)).ap()
            for i in range(8)]

        s_gt = nc.alloc_semaphore("s_gt")
        s_l0 = [nc.alloc_semaphore(f"s_l0{i}") for i in range(3)]
        s_ld = [[nc.alloc_semaphore(f"s_ld{h}_{cb}") for cb in range(NCB)]
                for h in range(2)]
        s_mm = nc.alloc_semaphore("s_mm")
        s_cast = {"v": nc.alloc_semaphore("s_cast_v"),
                  "a": nc.alloc_semaphore("s_cast_a")}
        s_st = {"sync": nc.alloc_semaphore("s_st_sync"),
                "scalar": nc.alloc_semaphore("s_st_sca")}
        all_sems = ([s_gt, s_mm, s_cast["v"], s_cast["a"],
                     s_st["sync"], s_st["scalar"]] + s_l0
                    + [s for row in s_ld for s in row])

        def emit_load(sync, img, cb):
            # one contiguous-2D DMA: [128 part, 3136 cols]
            if img >= 2:
                sync.wait_ge(s_mm, TILES_PER_IMG * (img - 2)
                             + TILES_PER_CB * (cb + 1))
            sync.dma_start(
                a_sb[img % 2][:, cb * CBW:(cb + 1) * CBW],
                act.ap()[img, :, cb * CBW:(cb + 1) * CBW],
            ).then_inc(s_ld[img % 2][cb], 16)

        def emit_store(eng, ring, img, zc, h2):
            # half-image-width store: cbs {2*h2, 2*h2+1}
            e = CAST_ENG[zc]
            eng.wait_ge(s_cast[e],
                        _cnt[e][POS[(img, 2 * h2 + 1, zc, P2 - 1)]])
            eng.dma_start(
                out[zc].ap()[img, :, h2 * 2 * CB:(h2 + 1) * 2 * CB],
                o_sb[img][zc][:, h2 * 2 * CB:(h2 + 1) * 2 * CB],
            ).then_inc(s_st[ring], 16)

        # Stage 0: clear semaphores, then prefetch gt + (img0, cb0) on the
        # sync queue (ordered after the clears on that queue; their sem
        # increments land while the main block is still in its barrier).
        # Sems are NOT zeroed on alloc and must not carry values across
        # executions, hence the clears.
        with nc.Block("clears") as blk:

            @blk.sync
            def _(sync):
                # clear only the prefetch sems before the doorbell; the
                # rest clear while the first transfers are in flight (their
                # first increments only happen deep into the main block)
                for s in s_l0:
                    sync.sem_clear(s)
                # (img0, cb0) in three pieces (jc{0,1} | jc2 | jc3) so the
                # PE starts on zc1 (which reads only jc{0,1}) while the rest
                # is in flight; zc2 needs jc2, zc3/zc0 need jc3
                for i, (c0, c1) in enumerate(
                        ((0, 2 * CB), (2 * CB, 3 * CB), (3 * CB, CBW))):
                    sync.dma_start(
                        a_sb[0][:, c0:c1],
                        act.ap()[0, :, c0:c1],
                    ).then_inc(s_l0[i], 16)
                for s in all_sems:
                    if s not in s_l0 and s is not s_gt:
                        sync.sem_clear(s)

            @blk.scalar
            def _(scalar):
                # gt rides the otherwise-idle scalar ring so its receipt
                # doesn't queue behind the activation packets
                scalar.sem_clear(s_gt)
                scalar.dma_start(gt_sb[:], gtp.ap()[:]).then_inc(s_gt, 16)

            @blk.tensor
            def _(tensor):
                # HAM warmup on junk SBUF while the clears + first loads
                # issue: the PE's clock gate needs ~3.4us of sustained
                # activity to lift the 1.2GHz cold throttle, so burn that
                # window here where the tensor engine would idle anyway
                # (start=True resets bank 7 before its first real use).
                for _ in range(28):
                    tensor.matmul(psum[7][:, :P], a_sb[0][:, :P],
                                  a_sb[0][:, :P], start=True, stop=True)

        LAST = IMGS - 1  # last image: finer stores, split across both rings

        with nc.Block("main") as blk:

            @blk.sync
            def _(sync):
                for cb in range(1, NCB):
                    emit_load(sync, 0, cb)
                for cb in range(NCB):
                    emit_load(sync, 1, cb)
                n_store = 0
                for img in range(IMGS):
                    for h2 in range(NCB // 2):
                        if img + 2 < IMGS:
                            emit_load(sync, img + 2, 2 * h2)
                            emit_load(sync, img + 2, 2 * h2 + 1)
                        for zc in range(NCHUNK):
                            if img == LAST and CAST_ENG[zc] == "a":
                                continue  # on the scalar ring
                            if img == LAST and h2 == 1:
                                continue  # finer drain stores below
                            emit_store(sync, "sync", img, zc, h2)
                            n_store += 1
                # drain: per-column-block stores for the last image's final
                # half so the closing transfer+receipt is as short as
                # possible; sync also takes zc2 (an ACT-cast chunk) so the
                # scalar queue only drains zc3 behind its final casts
                for cb in (2, 3):
                    for zc in (0, 1, 2):
                        e = CAST_ENG[zc]
                        sync.wait_ge(s_cast[e],
                                     _cnt[e][POS[(LAST, cb, zc, P2 - 1)]])
                        sync.dma_start(
                            out[zc].ap()[LAST, :, cb * CB:(cb + 1) * CB],
                            o_sb[LAST][zc][:, cb * CB:(cb + 1) * CB],
                        ).then_inc(s_st["sync"], 16)
                        n_store += 1
                sync.wait_ge(s_st["sync"], 16 * n_store)

            @blk.scalar
            def _(scalar):
                n_store = 0
                for q, (img, cb, zc, p2) in enumerate(TILE_SEQ):
                    if CAST_ENG[zc] != "a":
                        continue
                    scalar.wait_ge(s_mm, q + 1)
                    p = cb * P2 + p2
                    scalar.copy(
                        o_sb[img][zc][:, p * PT:(p + 1) * PT],
                        psum[q % 8][:, :PT],
                    ).then_inc(s_cast["a"])
                    # last image, first half: store right here (slack in the
                    # cast chain); the final half's stores are deferred below
                    # so the last casts aren't delayed by store issue
                    if (img == LAST and cb == 1 and p2 == P2 - 1):
                        emit_store(scalar, "scalar", img, zc, 0)
                        n_store += 1
                # drain: last-half zc3 stores, per column block, after all
                # casts (waits already satisfied; zc2's went to sync)
                for cb in (2, 3):
                    zc = 3
                    scalar.dma_start(
                        out[zc].ap()[LAST, :, cb * CB:(cb + 1) * CB],
                        o_sb[LAST][zc][:, cb * CB:(cb + 1) * CB],
                    ).then_inc(s_st["scalar"], 16)
                    n_store += 1
                scalar.wait_ge(s_st["scalar"], 16 * n_store)

            @blk.tensor
            def _(tensor):
                # bridge the warmup across the block barrier: keep the PE
                # busy until the first loads' semaphores land, else the
                # ~2.5us idle gap re-arms the cold throttle
                for _ in range(4):
                    tensor.matmul(psum[7][:, :P], a_sb[0][:, :P],
                                  a_sb[0][:, :P], start=True, stop=True)
                tensor.wait_ge(s_gt, 16)
                for img in range(IMGS):
                    for cb in range(NCB):
                        if (img, cb) != (0, 0):
                            # loads on s_ld[h][cb] from imgs of the same
                            # parity, excluding (0,0) which used s_l0
                            n_prior = sum(
                                1 for i in range(img % 2, img + 1, 2)
                                if (i, cb) != (0, 0))
                            tensor.wait_ge(s_ld[img % 2][cb], 16 * n_prior)
                        for zc in _zc_order(img, cb):
                            if (img, cb) == (0, 0):
                                if zc in (1, 2):
                                    tensor.wait_ge(s_l0[zc - 1], 16)
                                elif zc == 3:
                                    tensor.wait_ge(s_l0[2], 16)
                            for p2 in range(P2):
                                q = POS[(img, cb, zc, p2)]
                                if q >= 8:
                                    e = CAST_ENG[TILE_SEQ[q - 8][2]]
                                    tensor.wait_ge(s_cast[e],
                                                   _cnt[e][q - 8])
                                for i, d in enumerate(KEPT_D):
                                    jc = (zc - d) % NCHUNK
                                    c0 = cb * CBW + jc * CB + p2 * PT
                                    mm = tensor.matmul(
                                        psum[q % 8][:, :PT],
                                        gt_sb[:, (zc * NKEPT + i) * P:
                                              (zc * NKEPT + i + 1) * P],
                                        a_sb[img % 2][:, c0:c0 + PT],
                                        start=(i == 0), stop=(i == NKEPT - 1),
                                    )
                                mm.then_inc(s_mm)

            @blk.vector
            def _(vector):
                for q, (img, cb, zc, p2) in enumerate(TILE_SEQ):
                    if CAST_ENG[zc] != "v":
                        continue
                    vector.wait_ge(s_mm, q + 1)
                    p = cb * P2 + p2
                    vector.tensor_copy(
                        o_sb[img][zc][:, p * PT:(p + 1) * PT],
                        psum[q % 8][:, :PT],
                    ).then_inc(s_cast["v"])

    nc.compile()
    return nc


def _make_gt(inhib_kernel: np.ndarray) -> np.ndarray:
    """Packed stationary blocks: col block (zc*NKEPT+i) holds
    GTs[jc*P:(jc+1)*P, zc*P:(zc+1)*P] with jc=(zc-KEPT_D[i])%NCHUNK,
    where GTs[j, r] = g[(r + ROT - j) mod C]."""
    k = np.asarray(inhib_kernel, dtype=np.float64)
    g = np.real(np.fft.ifft(1.0 / np.fft.fft(k)))
    gts = g[(np.arange(C)[None, :] + ROT - np.arange(C)[:, None]) % C]
    gtp = np.empty((P, NCHUNK * NKEPT * P), dtype=NP_OF[W_DT])
    for zc in range(NCHUNK):
        for i, d in enumerate(KEPT_D):
            jc = (zc - d) % NCHUNK
            b = zc * NKEPT + i
            gtp[:, b * P:(b + 1) * P] = gts[jc * P:(jc + 1) * P,
                                            zc * P:(zc + 1) * P]
    return np.ascontiguousarray(gtp)


def make_in_maps(activations, inhib_kernel):
    acts = np.asarray(activations, dtype=np.float32).reshape(N, C, HW)
    acts8 = acts.astype(NP_OF[A_DT])
    # [n, (jc p), (cb m)] -> [n, p, (cb jc m)]: partition-major, cb-major
    # so each (img, cb) device load is one fully contiguous 2D transfer
    acts8 = acts8.reshape(N, NCHUNK, P, NCB, CB).transpose(0, 2, 3, 1, 4)
    acts8 = np.ascontiguousarray(acts8).reshape(N, P, NCHUNK * HW)
    gtp = _make_gt(np.asarray(inhib_kernel))
    return [
        {"act": acts8[c * IMGS:(c + 1) * IMGS], "gtp": gtp}
        for c in range(N_CORES)
    ]


def kernel(activations, inhib_kernel):
    acts = np.asarray(activations, dtype=np.float32)
    assert acts.shape == (N, C, H, W), acts.shape

    if "nc" not in _CACHE:
        _CACHE["nc"] = _build_nc()
    nc = _CACHE["nc"]

    in_maps = make_in_maps(acts, inhib_kernel)
    res = run_bass_kernel_spmd(nc, in_maps, core_ids=list(range(N_CORES)))
    z = np.concatenate(
        [np.concatenate([r[f"out{zc}"].astype(np.float32)
                         for zc in range(NCHUNK)], axis=1)
         for r in res.results], axis=0)
    # un-rotate: y[i] = z[(i - ROT) mod C]
    y = z[:, (np.arange(C) - ROT) % C, :]
    return y.reshape(N, C, H, W)


# revision 40
# speedup vs baseline: 1.0232x; 1.0232x over previous
"""ConvergedInhibition TRN2 kernel.

The reference computes, per pixel (n,h,w), an FFT deconvolution along the
channel axis: y = ifft(fft(x)/fft(k)).real. Since k is fixed, this is a
circular convolution with g = ifft(1/fft(k)): y[i] = sum_j g[(i-j) mod C] x[j]
— a dense CxC circulant matmul applied to every pixel. Viewing activations[n]
as a [C, H*W] matrix A_n, the problem is out_n = G @ A_n: a [512,512] x
[512,3136] matmul per image, data-parallel over 32 images across 8 cores.

Implementation choices (measured on HW):
- The deconv kernel g is concentrated in a ~224-wide circular window.
  Rotating output rows by S=288 (z[r] = y[(r+S) mod C]) aligns the support
  so only 2 of 4 K-chunks of the contraction carry mass (each output row
  keeps a 256-wide sliding window of g; truncation costs ~2e-3 rel).
  The rotation is undone by a host-side gather.
- fp8 (e3m4) I/O: |x| < 6 << 15.5 = e3m4 max, 4 mantissa bits -> ~1.34e-2
  rms rounding per side (measured end-to-end rel err 1.907e-2, HW matches
  the numpy simulation exactly). Per-zc output dtype stays configurable.
  Weights are fp16 (PE upcasts operands to FP22, mixed dtypes allowed).
- Only the 8 needed [128,128] weight blocks ship (256 KB, one DMA on the
  otherwise-idle scalar ring).
- Each dma_start occupies its HWDGE ring ~630ns regardless of size, so
  DMAs are as large as possible: the host pre-arranges activations
  partition-major/cb-major so each (img, cb) load is one fully contiguous
  400KB 2D transfer; stores are half-image-width. Loads + most stores ride
  the sync ring; the last image's zc3 stores drain on the scalar ring.
- Startup: the first (img0, cb0) load is prefetched from the semaphore-
  clears block in three pieces (jc{0,1} | jc2 | jc3) and that block's tile
  order is zc [1,2,3,0], so real matmuls start on the first piece while
  the rest is in flight. Junk warmup matmuls bridge from the clears block
  to first data so the PE's HAM clock gate (1.2GHz when cold, ~3.4us of
  activity to lift) is fully open before real work; without this, any
  >2.5us PE idle gap re-arms the throttle mid-kernel.
- PSUM->out casts are split between DVE (zc 0,1) and ACT (zc 2,3), each
  with its own completion semaphore (PSUM bank = seq position % 8; the
  bank-reuse wait looks up the engine of the tile 8 positions back), so
  cast rate keeps up with the PE.
- Measured engine budget per core: PE window 42.6us (256 x 166ns, zero
  stalls), DVE 64 casts x 558ns, ACT 64 x 587ns, ~13.1MB HBM traffic.
"""

import numpy as np
import ml_dtypes

import concourse.bass as bass  # noqa: F401  (registers bass types)
import concourse.mybir as mybir
from concourse import bacc
from concourse.bass_utils import run_bass_kernel_spmd

N_CORES = 8
N, C, H, W = 32, 512, 56, 56
HW = H * W                      # 3136
IMGS = N // N_CORES             # 4 images per core
P = 128                         # partitions
NCHUNK = C // P                 # 4
PT = 392                        # pixel tile (free dim), 3136 = 8*392
NPT = HW // PT                  # 8
CB = 784                        # column block, 3136 = 4*784
NCB = HW // CB                  # 4
ROT = 288                       # output-row rotation aligning g's support
KEPT_D = (0, 1)                 # kept (zc - jc) mod 4 chunk distances
NKEPT = len(KEPT_D)

F8 = mybir.dt.float8e3
F16 = mybir.dt.float16
NP_OF = {F8: ml_dtypes.float8_e3m4, F16: np.float16}
A_DT = F8                       # activation (input) dtype
OUT_DT = [F8, F8, F8, F8]       # output dtype per output chunk zc
W_DT = F16                      # gt weights
CAST_ENG = ["v", "v", "a", "a"]  # cast engine per zc (v=DVE, a=ACT)

P2 = NPT // NCB                       # p-tiles per column block (2)
TILES_PER_CB = NCHUNK * P2            # 8 psum tiles per (img, cb)
TILES_PER_IMG = NCB * TILES_PER_CB    # 32
NTILES = IMGS * TILES_PER_IMG         # 128


# Tile execution sequence: position in TILE_SEQ is the tile id (PSUM bank =
# pos % 8, s_mm counts positions). (img0, cb0) runs zc order [1,2,3,0] so
# compute can start on the first half-load (zc1 reads only jc{0,1}; zc2/zc3
# only need the second half; zc0 wraps to jc3).
def _zc_order(img, cb):
    # (0,0): follow the three-piece first load (zc1 needs only jc{0,1}...).
    # Elsewhere: alternate the cast engines (v,v,a,a,v,v,a,a) so both
    # engines' cast chains pipeline with the matmul stream instead of one
    # engine's four casts bunching up after the last matmul of the block.
    return (1, 2, 3, 0) if (img, cb) == (0, 0) else (0, 2, 1, 3)


TILE_SEQ = []                      # (img, cb, zc, p2) in execution order
for _img in range(IMGS):
    for _cb in range(NCB):
        for _zc in _zc_order(_img, _cb):
            for _p2 in range(P2):
                TILE_SEQ.append((_img, _cb, _zc, _p2))
POS = {k: q for q, k in enumerate(TILE_SEQ)}

# cnt_eng[e][q]: number of tiles at positions <= q cast by engine e
_cnt = {"v": [0] * NTILES, "a": [0] * NTILES}
_c = {"v": 0, "a": 0}
for _q, (_i, _b, _zc, _p) in enumerate(TILE_SEQ):
    _c[CAST_ENG[_zc]] += 1
    _cnt["v"][_q] = _c["v"]
    _cnt["a"][_q] = _c["a"]

_CACHE = {}


def _build_nc():
    nc = bacc.Bacc("TRN2", target_bir_lowering=False, debug=False,
                   num_devices=N_CORES)
    # host pre-arranges activations partition-major and cb-major
    # ([img, p, cb, jc, m] flattened) so every (img, cb) load is a single
    # 2D DMA with 3136B-contiguous runs on both the HBM and SBUF side
    act = nc.dram_tensor("act", [IMGS, P, NCHUNK * HW], A_DT,
                         kind="ExternalInput")
    gtp = nc.dram_tensor("gtp", [P, NCHUNK * NKEPT * P], W_DT,
                         kind="ExternalInput")
    out = [nc.dram_tensor(f"out{zc}", [IMGS, P, HW], OUT_DT[zc],
                          kind="ExternalOutput") for zc in range(NCHUNK)]

    CBW = NCHUNK * CB  # 3136 cols per (cb) block in the cb-major layout

    from contextlib import ExitStack
    with ExitStack() as ctx:
        a_sb = [ctx.enter_context(
            nc.sbuf_tensor(f"a_sb{h}", [P, NCHUNK * HW], A_DT)).ap()
            for h in range(2)]
        gt_sb = ctx.enter_context(
            nc.sbuf_tensor("gt_sb", [P, NCHUNK * NKEPT * P], W_DT)).ap()
        o_sb = [[ctx.enter_context(
            nc.sbuf_tensor(f"o_sb{i}_{z}", [P, HW], OUT_DT[z])).ap()
            for z in range(NCHUNK)] for i in range(IMGS)]
        psum = [ctx.enter_context(
            nc.psum_tensor(f"ps{i}", [P, 512], mybir.dt.float32)).ap()
            for i in range(8)]

        s_gt = nc.alloc_semaphore("s_gt")
        s_l0 = [nc.alloc_semaphore(f"s_l0{i}") for i in range(3)]
        s_ld = [[nc.alloc_semaphore(f"s_ld{h}_{cb}") for cb in range(NCB)]
                for h in range(2)]
        s_mm = nc.alloc_semaphore("s_mm")
        s_cast = {"v": nc.alloc_semaphore("s_cast_v"),
                  "a": nc.alloc_semaphore("s_cast_a")}
        s_st = {"sync": nc.alloc_semaphore("s_st_sync"),
                "scalar": nc.alloc_semaphore("s_st_sca")}
        all_sems = ([s_gt, s_mm, s_cast["v"], s_cast["a"],
                     s_st["sync"], s_st["scalar"]] + s_l0
                    + [s for row in s_ld for s in row])

        def emit_load(sync, img, cb):
            # one contiguous-2D DMA: [128 part, 3136 cols]
            if img >= 2:
                sync.wait_ge(s_mm, TILES_PER_IMG * (img - 2)
                             + TILES_PER_CB * (cb + 1))
            sync.dma_start(
                a_sb[img % 2][:, cb * CBW:(cb + 1) * CBW],
                act.ap()[img, :, cb * CBW:(cb + 1) * CBW],
            ).then_inc(s_ld[img % 2][cb], 16)

        def emit_store(eng, ring, img, zc, h2):
            # half-image-width store: cbs {2*h2, 2*h2+1}
            e = CAST_ENG[zc]
            eng.wait_ge(s_cast[e],
                        _cnt[e][POS[(img, 2 * h2 + 1, zc, P2 - 1)]])
            eng.dma_start(
                out[zc].ap()[img, :, h2 * 2 * CB:(h2 + 1) * 2 * CB],
                o_sb[img][zc][:, h2 * 2 * CB:(h2 + 1) * 2 * CB],
            ).then_inc(s_st[ring], 16)

        # Stage 0: clear semaphores, then prefetch gt + (img0, cb0) on the
        # sync queue (ordered after the clears on that queue; their sem
        # increments land while the main block is still in its barrier).
        # Sems are NOT zeroed on alloc and must not carry values across
        # executions, hence the clears.
        with nc.Block("clears") as blk:

            @blk.sync
            def _(sync):
                for s in all_sems:
                    sync.sem_clear(s)
                # (img0, cb0) in three pieces (jc{0,1} | jc2 | jc3) so the
                # PE starts on zc1 (which reads only jc{0,1}) while the rest
                # is in flight; zc2 needs jc2, zc3/zc0 need jc3
                for i, (c0, c1) in enumerate(
                        ((0, 2 * CB), (2 * CB, 3 * CB), (3 * CB, CBW))):
                    sync.dma_start(
                        a_sb[0][:, c0:c1],
                        act.ap()[0, :, c0:c1],
                    ).then_inc(s_l0[i], 16)

            @blk.scalar
            def _(scalar):
                # gt rides the otherwise-idle scalar ring so its receipt
                # doesn't queue behind the activation packets
                scalar.dma_start(gt_sb[:], gtp.ap()[:]).then_inc(s_gt, 16)

            @blk.tensor
            def _(tensor):
                # HAM warmup on junk SBUF while the clears + first loads
                # issue: the PE's clock gate needs ~3.4us of sustained
                # activity to lift the 1.2GHz cold throttle, so burn that
                # window here where the tensor engine would idle anyway
                # (start=True resets bank 7 before its first real use).
                for _ in range(28):
                    tensor.matmul(psum[7][:, :P], a_sb[0][:, :P],
                                  a_sb[0][:, :P], start=True, stop=True)

        LAST = IMGS - 1  # last image: finer stores, split across both rings

        with nc.Block("main") as blk:

            @blk.sync
            def _(sync):
                for cb in range(1, NCB):
                    emit_load(sync, 0, cb)
                for cb in range(NCB):
                    emit_load(sync, 1, cb)
                n_store = 0
                for img in range(IMGS):
                    for h2 in range(NCB // 2):
                        if img + 2 < IMGS:
                            emit_load(sync, img + 2, 2 * h2)
                            emit_load(sync, img + 2, 2 * h2 + 1)
                        for zc in range(NCHUNK):
                            if img == LAST and CAST_ENG[zc] == "a":
                                continue  # on the scalar ring
                            if img == LAST and h2 == 1:
                                continue  # finer drain stores below
                            emit_store(sync, "sync", img, zc, h2)
                            n_store += 1
                # drain: per-column-block stores for the last image's final
                # half so the closing transfer+receipt is as short as
                # possible; sync also takes zc2 (an ACT-cast chunk) so the
                # scalar queue only drains zc3 behind its final casts
                for cb in (2, 3):
                    for zc in (0, 1, 2):
                        e = CAST_ENG[zc]
                        sync.wait_ge(s_cast[e],
                                     _cnt[e][POS[(LAST, cb, zc, P2 - 1)]])
                        sync.dma_start(
                            out[zc].ap()[LAST, :, cb * CB:(cb + 1) * CB],
                            o_sb[LAST][zc][:, cb * CB:(cb + 1) * CB],
                        ).then_inc(s_st["sync"], 16)
                        n_store += 1
                sync.wait_ge(s_st["sync"], 16 * n_store)

            @blk.scalar
            def _(scalar):
                n_store = 0
                for q, (img, cb, zc, p2) in enumerate(TILE_SEQ):
                    if CAST_ENG[zc] != "a":
                        continue
                    scalar.wait_ge(s_mm, q + 1)
                    p = cb * P2 + p2
                    scalar.copy(
                        o_sb[img][zc][:, p * PT:(p + 1) * PT],
                        psum[q % 8][:, :PT],
                    ).then_inc(s_cast["a"])
                    # last image, first half: store right here (slack in the
                    # cast chain); the final half's stores are deferred below
                    # so the last casts aren't delayed by store issue
                    if (img == LAST and cb == 1 and p2 == P2 - 1):
                        emit_store(scalar, "scalar", img, zc, 0)
                        n_store += 1
                # drain: last-half zc3 stores, per column block, after all
                # casts (waits already satisfied; zc2's went to sync)
                for cb in (2, 3):
                    zc = 3
                    scalar.dma_start(
                        out[zc].ap()[LAST, :, cb * CB:(cb + 1) * CB],
                        o_sb[LAST][zc][:, cb * CB:(cb + 1) * CB],
                    ).then_inc(s_st["scalar"], 16)
                    n_store += 1
                scalar.wait_ge(s_st["scalar"], 16 * n_store)

            @blk.tensor
            def _(tensor):
                # bridge the warmup across the block barrier: keep the PE
                # busy until the first loads' semaphores land, else the
                # ~2.5us idle gap re-arms the cold throttle
                for _ in range(4):
                    tensor.matmul(psum[7][:, :P], a_sb[0][:, :P],
                                  a_sb[0][:, :P], start=True, stop=True)
                tensor.wait_ge(s_gt, 16)
                for img in range(IMGS):
                    for cb in range(NCB):
                        if (img, cb) != (0, 0):
                            # loads on s_ld[h][cb] from imgs of the same
                            # parity, excluding (0,0) which used s_l0
                            n_prior = sum(
                                1 for i in range(img % 2, img + 1, 2)
                                if (i, cb) != (0, 0))
                            tensor.wait_ge(s_ld[img % 2][cb], 16 * n_prior)
                        for zc in _zc_order(img, cb):
                            if (img, cb) == (0, 0):
                                if zc in (1, 2):
                                    tensor.wait_ge(s_l0[zc - 1], 16)
                                elif zc == 3:
                                    tensor.wait_ge(s_l0[2], 16)
                            for p2 in range(P2):
                                q = POS[(img, cb, zc, p2)]
                                if q >= 8:
                                    e = CAST_ENG[TILE_SEQ[q - 8][2]]
                                    tensor.wait_ge(s_cast[e],
                                                   _cnt[e][q - 8])
                                for i, d in enumerate(KEPT_D):
                                    jc = (zc - d) % NCHUNK
                                    c0 = cb * CBW + jc * CB + p2 * PT
                                    mm = tensor.matmul(
                                        psum[q % 8][:, :PT],
                                        gt_sb[:, (zc * NKEPT + i) * P:
                                              (zc * NKEPT + i + 1) * P],
                                        a_sb[img % 2][:, c0:c0 + PT],
                                        start=(i == 0), stop=(i == NKEPT - 1),
                                    )
                                mm.then_inc(s_mm)

            @blk.vector
            def _(vector):
                for q, (img, cb, zc, p2) in enumerate(TILE_SEQ):
                    if CAST_ENG[zc] != "v":
                        continue
                    vector.wait_ge(s_mm, q + 1)
                    p = cb * P2 + p2
                    vector.tensor_copy(
                        o_sb[img][zc][:, p * PT:(p + 1) * PT],
                        psum[q % 8][:, :PT],
                    ).then_inc(s_cast["v"])

    nc.compile()
    return nc


def _make_gt(inhib_kernel: np.ndarray) -> np.ndarray:
    """Packed stationary blocks: col block (zc*NKEPT+i) holds
    GTs[jc*P:(jc+1)*P, zc*P:(zc+1)*P] with jc=(zc-KEPT_D[i])%NCHUNK,
    where GTs[j, r] = g[(r + ROT - j) mod C]."""
    k = np.asarray(inhib_kernel, dtype=np.float64)
    g = np.real(np.fft.ifft(1.0 / np.fft.fft(k)))
    gts = g[(np.arange(C)[None, :] + ROT - np.arange(C)[:, None]) % C]
    gtp = np.empty((P, NCHUNK * NKEPT * P), dtype=NP_OF[W_DT])
    for zc in range(NCHUNK):
        for i, d in enumerate(KEPT_D):
            jc = (zc - d) % NCHUNK
            b = zc * NKEPT + i
            gtp[:, b * P:(b + 1) * P] = gts[jc * P:(jc + 1) * P,
                                            zc * P:(zc + 1) * P]
    return np.ascontiguousarray(gtp)


def make_in_maps(activations, inhib_kernel):
    acts = np.asarray(activations, dtype=np.float32).reshape(N, C, HW)
    acts8 = acts.astype(NP_OF[A_DT])
    # [n, (jc p), (cb m)] -> [n, p, (cb jc m)]: partition-major, cb-major
    # so each (img, cb) device load is one fully contiguous 2D transfer
    acts8 = acts8.reshape(N, NCHUNK, P, NCB, CB).transpose(0, 2, 3, 1, 4)
    acts8 = np.ascontiguousarray(acts8).reshape(N, P, NCHUNK * HW)
    gtp = _make_gt(np.asarray(inhib_kernel))
    return [
        {"act": acts8[c * IMGS:(c + 1) * IMGS], "gtp": gtp}
        for c in range(N_CORES)
    ]


def kernel(activations, inhib_kernel):
    acts = np.asarray(activations, dtype=np.float32)
    assert acts.shape == (N, C, H, W), acts.shape

    if "nc" not in _CACHE:
        _CACHE["nc"] = _build_nc()
    nc = _CACHE["nc"]

    in_maps = make_in_maps(acts, inhib_kernel)
    res = run_bass_kernel_spmd(nc, in_maps, core_ids=list(range(N_CORES)))
    z = np.concatenate(
        [np.concatenate([r[f"out{zc}"].astype(np.float32)
                         for zc in range(NCHUNK)], axis=1)
         for r in res.results], axis=0)
    # un-rotate: y[i] = z[(i - ROT) mod C]
    y = z[:, (np.arange(C) - ROT) % C, :]
    return y.reshape(N, C, H, W)
